# revision 6
# baseline (speedup 1.0000x reference)
"""ChannelBlock Trainium2 Bass kernel v2, data-parallel over batch on 8 cores.

Design vs baseline:
- x / out DRAM layout [128, 32, 256]: partition p owns tokens p*32..p*32+31,
  so whole-x loads/stores are a few big contiguous-per-partition DMAs.
- All transposes via XBAR DMA-transpose (SBUF->SBUF), none on the PE.
- fp8 DoubleRow matmuls for kv, attention accumulation, attn-apply (y),
  fc1 and fc2 (weights scaled by 64; descale folded into evictions).
- q projection folded: out_attn = xhat @ F with F = Wq @ BD^T @ Wproj
  computed at runtime (4 small matmuls) - phase A2 of the baseline is gone.
- fc2 computed feature-major (weights stationary, tokens moving) and
  XBAR-transposed back - halves fc2 instruction count.
- C1 residual+LN2 stats fused via scalar_tensor_tensor accum_out.
"""

import os

import numpy as np

import concourse.bass as bass
import concourse.bass_utils as _bu
import concourse.tile as tile
from concourse import mybir
from concourse.bass_utils import run_bass_kernel_spmd
from concourse.vector_clock import ScopedClock
import bass_rust

# ----------------------------------------------------------------------------
# Workaround: this container's walrus only supports ONE sync-wait command per
# TPB_CTRL instruction; split Tile's tail drain into a chain of drains.
# ----------------------------------------------------------------------------
_MAX_DRAIN_WAITS = 1


def _patched_drain_and_barrier(self, tick_clock, wait_clock):
    drain_inst = self.nc.sync.drain()
    wait_clock.add_sem_waits(
        drain_inst.ins, ScopedClock({None: tick_clock.global_clock})
    )
    mi = drain_inst.ins
    si = mi.sync_info
    waits = list(si.on_wait) if si else []
    if len(waits) > _MAX_DRAIN_WAITS:
        mi.sync_info = bass_rust.SyncInfo(
            on_wait=waits[:_MAX_DRAIN_WAITS], on_update=list(si.on_update)
        )
        for i in range(_MAX_DRAIN_WAITS, len(waits), _MAX_DRAIN_WAITS):
            extra = self.nc.sync.drain()
            extra.ins.sync_info = bass_rust.SyncInfo(
                on_wait=waits[i : i + _MAX_DRAIN_WAITS], on_update=[]
            )
    self.nc.all_engine_barrier()
    popped = self.nc._tile_sem_poison_stack.pop()
    assert popped is self._sem_poison
    self.nc.clear_and_free_semaphores(list(self.sems.allocated().values()))
    self.nc.all_engine_barrier()


tile.TileContext._drain_and_barrier = _patched_drain_and_barrier

_nop_counter = [0]


def _split_sync_waits(nc, cap=1):
    """Hoist excess sync-waits onto same-engine NOPs (walrus accepts only
    `cap` waits per instruction; Ldweights may carry none)."""
    for f in nc.m.functions:
        for blk in f.blocks:
            changed = False
            new = []
            for inst in blk.instructions:
                si = inst.sync_info
                waits = list(si.on_wait) if si is not None else []
                is_ldw = inst.__class__.__name__ == "InstLdweights"
                eff_cap = 0 if (is_ldw and waits) else cap
                if len(waits) > eff_cap:
                    if is_ldw:
                        excess, keep = waits, []
                    else:
                        excess, keep = waits[:-cap], waits[-cap:]
                    for j in range(0, len(excess), cap):
                        _nop_counter[0] += 1
                        nop = mybir.InstNoOp(
                            name=f"NW-{_nop_counter[0]}", ins=[], outs=[]
                        )
                        nop.engine = inst.engine
                        nop.sync_info = bass_rust.SyncInfo(
                            on_wait=excess[j : j + cap], on_update=[]
                        )
                        new.append(nop)
                    inst.sync_info = bass_rust.SyncInfo(
                        on_wait=keep, on_update=list(si.on_update)
                    )
                    changed = True
                new.append(inst)
            if changed:
                blk.instructions = new


# ----------------------------------------------------------------------------
# Problem constants
# ----------------------------------------------------------------------------
B = 8
N = 4096
C = 256
H = 8
HD = C // H  # 32
HID = 1024
EPS = 1e-5
P = 128
NTILES = N // P  # 32
SCALE = HD ** (-0.5)

F32 = mybir.dt.float32
BF16 = mybir.dt.bfloat16
FP8 = mybir.dt.float8e4
NP_BF16 = mybir.dt.np(BF16)
NP_FP8 = mybir.dt.np(FP8)

WS = 64.0  # weight fp8 scale (wkv, w1, w2)
FS = 256.0  # F fp8 scale

AF = mybir.ActivationFunctionType
ALU = mybir.AluOpType
AX = mybir.AxisListType
DR = mybir.MatmulPerfMode.DoubleRow


def _act_rsqrt(nc, out, in_, bias):
    """activation(Rsqrt) without bass's accuracy guard (fine at 2e-2 tol):
    out = 1/sqrt(in + bias)."""
    eng = nc.scalar
    inputs = [eng.lower_ap(in_)]
    for arg in (bias, 1.0, 0.0):  # bias, scale, alpha
        if isinstance(arg, float):
            inputs.append(mybir.ImmediateValue(dtype=mybir.dt.float32, value=arg))
        else:
            inputs.append(eng.lower_ap(arg))
    return eng.add_instruction(
        mybir.InstActivation(
            name=nc.get_next_instruction_name(),
            func=AF.Rsqrt,
            ins=inputs,
            outs=[eng.lower_ap(out)],
        )
    )


def _build_nc(has_bkv, has_bproj, has_bq, has_bfc2):
    nc = bass.Bass()

    # ---- DRAM I/O ----
    x_d = nc.declare_dram_parameter("x", [P, NTILES, C], BF16, isOutput=False)
    wbf_d = nc.declare_dram_parameter("wbf", [P, 1024], BF16, isOutput=False)
    w8_d = nc.declare_dram_parameter("w8", [P, 5120], FP8, isOutput=False)
    bfc2_d = nc.declare_dram_parameter("bfc2", [1, C], BF16, isOutput=False)
    bias_d = nc.declare_dram_parameter("bias", [P, 12], F32, isOutput=False)
    bkv_d = nc.declare_dram_parameter("bkv", [1, 2 * C], BF16, isOutput=False)
    bproj_d = nc.declare_dram_parameter("bproj", [1, C], BF16, isOutput=False)
    out_d = nc.declare_dram_parameter("out", [P, NTILES, C], F32, isOutput=True)

    with tile.TileContext(nc) as tc:
        import contextlib

        ctx = contextlib.ExitStack()
        with ctx:
            const = ctx.enter_context(tc.tile_pool(name="const", bufs=1))
            xres = ctx.enter_context(tc.tile_pool(name="xres", bufs=1))
            stats = ctx.enter_context(tc.tile_pool(name="stats", bufs=4))
            work = ctx.enter_context(tc.tile_pool(name="work", bufs=4))
            kvp = ctx.enter_context(tc.tile_pool(name="kvp", bufs=5))
            tpb = ctx.enter_context(tc.tile_pool(name="tpb", bufs=4))
            big = ctx.enter_context(tc.tile_pool(name="bigbuf", bufs=4))
            outp = ctx.enter_context(tc.tile_pool(name="outp", bufs=3))
            ps_big = ctx.enter_context(
                tc.tile_pool(name="ps_big", bufs=3, space="PSUM")
            )
            ps_small = ctx.enter_context(
                tc.tile_pool(name="ps_small", bufs=3, space="PSUM")
            )
            ps_attn = ctx.enter_context(
                tc.tile_pool(name="ps_attn", bufs=1, space="PSUM")
            )

            # ---- weight / const SBUF residents ----
            wbf = const.tile([P, 1024], BF16)  # wproj(512) | wqT(512)
            w8 = const.tile([P, 5120], FP8)  # wkv(1024) | w1(2048) | w2(2048)
            biases = const.tile([P, 12], F32)  # bq(2) | b1(8) | bfc2(2)
            nc.scalar.dma_start(out=w8[:, 0:1024], in_=w8_d[:, 0:1024])
            nc.scalar.dma_start(out=wbf[:], in_=wbf_d[:])
            nc.scalar.dma_start(out=w8[:, 1024:5120], in_=w8_d[:, 1024:5120])
            nc.scalar.dma_start(out=biases[:], in_=bias_d[:])
            wkv8 = w8[:, 0:1024].rearrange("p (k j) -> p k j", k=2)
            wproj = wbf[:, 0:512].rearrange("p (h c) -> p h c", h=2)
            wqT = wbf[:, 512:1024].rearrange("p (h c) -> p h c", h=2)
            w1 = w8[:, 1024:3072].rearrange("p (k h) -> p k h", k=2)
            w2 = w8[:, 3072:5120].rearrange("p (h c) -> p h c", h=8)

            ones_row = const.tile([1, P], BF16)
            nc.vector.memset(ones_row[:], 1.0)
            eps_t = const.tile([P, 1], F32)
            nc.vector.memset(eps_t[:], EPS)
            bkv = const.tile([1, 2 * C], BF16)
            bproj = const.tile([1, C], BF16)
            bfc2 = const.tile([1, C], BF16)
            if has_bkv:
                nc.scalar.dma_start(out=bkv[:], in_=bkv_d[:])
            if has_bproj:
                nc.scalar.dma_start(out=bproj[:], in_=bproj_d[:])
            nc.scalar.dma_start(out=bfc2[:], in_=bfc2_d[:])

            # ---- residents ----
            x_sb = xres.tile([P, NTILES, C], BF16)  # x; becomes h1+x after C1
            h1_sb = xres.tile([P, NTILES, C], BF16)
            xhT8 = xres.tile([P, 2 * NTILES, P], FP8)  # LN1(x)^T, m = 2i+c
            # x input: big chunk loads (contiguous per partition)
            for lo, hi in ((0, 2), (2, 4), (4, 8), (8, 16), (16, 24), (24, 32)):
                nc.sync.dma_start(
                    out=x_sb[:, lo:hi, :], in_=x_d[:, lo:hi, :]
                )

            attn_ps = [
                ps_attn.tile([P, C], F32, name=f"attn_ps{i}") for i in range(2)
            ]
            def pe_warm(n_):
                # keep the PE p-state hot through otherwise idle stretches
                for w_ in range(n_):
                    wp = ps_big.tile([P, 2 * C], F32, tag="big", name=f"wm{_warm_ctr[0]}")
                    _warm_ctr[0] += 1
                    nc.tensor.matmul(
                        wp[:, 0:C], ones_row[:], warm_src[:],
                        start=True, stop=True,
                    )

            _warm_ctr = [0]

            warm_src = const.tile([1, C], BF16)
            nc.vector.memset(warm_src[:], 0.0)
            pe_warm(20)

            # =============== Phase A: LN1, xbar transpose, kv, attn ==========
            xtb_l = {}
            attn_cl = {}

            def a_head(g):
                idxs = [g * 4 + s for s in range(4)]
                stg = stats.tile([P, 4, 6], F32, tag="bn", name=f"bn{g}")
                mv4 = stats.tile([P, 4, 2], F32, tag="mv", name=f"mv{g}")
                for s, i in enumerate(idxs):
                    nc.vector.bn_stats(out=stg[:, s, :], in_=x_sb[:, i, :])
                    nc.vector.bn_aggr(out=mv4[:, s, :], in_=stg[:, s, :])
                rs4 = stats.tile([P, 4], F32, tag="rs", name=f"rs{g}")
                _act_rsqrt(nc, rs4[:], mv4[:, :, 1], eps_t[:])
                nmr4 = stats.tile([P, 4], F32, tag="nmr", name=f"nm{g}")
                nc.vector.scalar_tensor_tensor(
                    out=nmr4[:],
                    in0=mv4[:, :, 0],
                    scalar=-1.0,
                    in1=rs4[:],
                    op0=ALU.mult,
                    op1=ALU.mult,
                )
                xh4 = work.tile([P, 4, C], BF16, tag="xh4", name=f"xh{g}")
                for s, i in enumerate(idxs):
                    nc.scalar.activation(
                        out=xh4[:, s, :],
                        in_=x_sb[:, i, :],
                        func=AF.Identity,
                        scale=rs4[:, s : s + 1],
                        bias=nmr4[:, s : s + 1],
                    )
                xtb = tpb.tile([P, 8, P], BF16, tag="xtb")
                nc.sync.dma_start_transpose(out=xtb[:], in_=xh4[:])
                xtb_l[g] = xtb

            def a_mid(g):
                idxs = [g * 4 + s for s in range(4)]
                nc.vector.tensor_copy(
                    out=xhT8[:, g * 8 : (g + 1) * 8, :], in_=xtb_l.pop(g)[:]
                )
                cls = []
                for s, i in enumerate(idxs):
                    pair = i // 2
                    jj = i % 2
                    if jj == 0:
                        kv8 = kvp.tile([P, 2, 2 * C], FP8, tag="kv", name=f"kv{i}")
                    kv_ps = ps_big.tile([P, 2 * C], F32, tag="big")
                    nc.tensor.matmul(
                        kv_ps[:],
                        xhT8[:, 2 * i : 2 * i + 2, :],
                        wkv8[:, :, :],
                        start=True,
                        stop=not has_bkv,
                        perf_mode=DR,
                    )
                    if has_bkv:
                        nc.tensor.matmul(
                            kv_ps[:], ones_row[:], bkv[:], start=False, stop=True
                        )
                    nc.vector.tensor_scalar(
                        out=kv8[:, jj, 0:C],
                        in0=kv_ps[:, 0:C],
                        scalar1=1.0 / WS,
                        scalar2=None,
                        op0=ALU.mult,
                    )
                    nc.scalar.mul(
                        out=kv8[:, jj, C : 2 * C],
                        in_=kv_ps[:, C : 2 * C],
                        mul=1.0 / WS,
                    )
                    if jj == 1:
                        def _attn(kv8=kv8, pair=pair):
                            for half in range(2):
                                nc.tensor.matmul(
                                    attn_ps[half][:, :],
                                    kv8[:, :, half * P : (half + 1) * P],
                                    kv8[:, :, C : 2 * C],
                                    start=(pair == 0),
                                    stop=(pair == NTILES // 2 - 1),
                                    perf_mode=DR,
                                )
                        cls.append(_attn)
                attn_cl[g] = cls

            def a_attn(g):
                for fn_ in attn_cl.pop(g):
                    fn_()

            NG = NTILES // 4
            for g in range(NG):
                a_head(g)
                a_mid(g)
                if g >= 1:
                    a_attn(g - 1)
            a_attn(NG - 1)

            # =============== Phase B: softmax, E, F =========================
            pe_warm(12)
            BdT = const.tile([P, 2, P], BF16)
            nc.vector.memset(BdT[:], 0.0)
            a2 = work.tile([P, 2, HD], F32, tag="attn")
            ex2 = work.tile([P, 2, HD], F32, tag="exps")
            for half in range(2):
                for h in range(4):
                    hh = half * 4 + h
                    nc.vector.tensor_copy(
                        out=a2[h * HD : (h + 1) * HD, half, :],
                        in_=attn_ps[half][
                            h * HD : (h + 1) * HD, hh * HD : (hh + 1) * HD
                        ],
                    )
            # logits are O(+-8): exp is safe without max subtraction
            nc.scalar.activation(
                out=ex2[:], in_=a2[:], func=AF.Exp, scale=SCALE
            )
            ssum = stats.tile([P, 2], F32, tag="ssum")
            nc.vector.tensor_reduce(
                out=ssum[:], in_=ex2[:], axis=AX.X, op=ALU.add
            )
            rec = stats.tile([P, 2], F32, tag="rec")
            nc.vector.reciprocal(out=rec[:], in_=ssum[:])
            for half in range(2):
                attn_n = work.tile([P, HD], F32, tag="attn_n", name=f"an{half}")
                nc.vector.tensor_scalar(
                    out=attn_n[:],
                    in0=ex2[:, half, :],
                    scalar1=rec[:, half : half + 1],
                    scalar2=None,
                    op0=ALU.mult,
                )
                for h in range(4):
                    nc.gpsimd.tensor_copy(
                        out=BdT[h * HD : (h + 1) * HD, half, h * HD : (h + 1) * HD],
                        in_=attn_n[h * HD : (h + 1) * HD, :],
                    )

            # E[half] = BdT[half]^T @ Wproj[half]  (bf16)
            E_sb = const.tile([P, 2, C], BF16)
            for half in range(2):
                e_ps = ps_small.tile([P, C], F32, tag="small")
                nc.tensor.matmul(
                    e_ps[:], BdT[:, half, :], wproj[:, half, :], start=True, stop=True
                )
                nc.vector.tensor_copy(out=E_sb[:, half, :], in_=e_ps[:])
            # F[kc] = sum_fh WqT[fh][:,kc]^T @ E[fh]  -> fp8 x FS
            F8 = const.tile([P, 2, C], FP8)
            for kc in range(2):
                f_ps = ps_small.tile([P, C], F32, tag="small")
                for fh in range(2):
                    nc.tensor.matmul(
                        f_ps[:],
                        wqT[:, fh, kc * P : (kc + 1) * P],
                        E_sb[:, fh, :],
                        start=(fh == 0),
                        stop=(fh == 1),
                    )
                nc.scalar.mul(out=F8[:, kc, :], in_=f_ps[:], mul=FS)
            G_sb = const.tile([1, C], BF16)
            if has_bq:
                bq_bf = const.tile([P, 2], BF16)
                nc.vector.tensor_copy(out=bq_bf[:], in_=biases[:, 0:2])
                g_ps = ps_small.tile([1, C], F32, tag="gsm")
                for fh in range(2):
                    nc.tensor.matmul(
                        g_ps[:],
                        bq_bf[:, fh : fh + 1],
                        E_sb[:, fh, :],
                        start=(fh == 0),
                        stop=(fh == 1),
                    )
                nc.scalar.mul(out=G_sb[:], in_=g_ps[:], mul=FS)

            # =============== Phase C1: y=xhat@F, residual, LN2 stats ========
            sums = xres.tile([P, NTILES], F32)
            ssqs = xres.tile([P, NTILES], F32)
            rs32 = xres.tile([P, NTILES], F32)
            mu32 = xres.tile([P, NTILES], F32)
            nmr32 = xres.tile([P, NTILES], F32)

            def c1_tile(i):
                p_ps = ps_small.tile([P, C], F32, tag="small", name=f"pp{i}")
                more = has_bproj or has_bq
                nc.tensor.matmul(
                    p_ps[:],
                    xhT8[:, 2 * i : 2 * i + 2, :],
                    F8[:, :, :],
                    start=True,
                    stop=not more,
                    perf_mode=DR,
                )
                if has_bq:
                    nc.tensor.matmul(
                        p_ps[:], ones_row[:], G_sb[:],
                        start=False, stop=not has_bproj,
                    )
                if has_bproj:
                    nc.tensor.matmul(
                        p_ps[:], ones_row[:], bproj[:], start=False, stop=True
                    )
                nc.vector.scalar_tensor_tensor(
                    out=h1_sb[:, i, :],
                    in0=p_ps[:],
                    scalar=1.0 / FS,
                    in1=x_sb[:, i, :],
                    op0=ALU.mult,
                    op1=ALU.add,
                    accum_out=sums[:, i : i + 1],
                )
                sq = work.tile([P, C], BF16, tag="sq", name=f"sq{i}")
                if i % 2 == 0:
                    nc.vector.scalar_tensor_tensor(
                        out=sq[:],
                        in0=h1_sb[:, i, :],
                        scalar=1.0,
                        in1=h1_sb[:, i, :],
                        op0=ALU.mult,
                        op1=ALU.mult,
                        accum_out=ssqs[:, i : i + 1],
                    )
                else:
                    nc.scalar.activation(
                        out=sq[:],
                        in_=h1_sb[:, i, :],
                        func=AF.Square,
                        accum_out=ssqs[:, i : i + 1],
                    )
                # rx = h1 + x, overwrites x slice (dead after this)
                nc.gpsimd.tensor_tensor(
                    out=x_sb[:, i, :],
                    in0=h1_sb[:, i, :],
                    in1=x_sb[:, i, :],
                    op=ALU.add,
                )

            def rstd8(b):
                sl = slice(b * 16, (b + 1) * 16)
                nc.vector.tensor_scalar(
                    out=mu32[:, sl], in0=sums[:, sl], scalar1=1.0 / C,
                    scalar2=None, op0=ALU.mult,
                )
                var8 = stats.tile([P, 16], F32, tag="var8", name=f"va{b}")
                nc.vector.tensor_tensor(
                    out=var8[:], in0=mu32[:, sl], in1=mu32[:, sl], op=ALU.mult
                )
                nc.vector.scalar_tensor_tensor(
                    out=var8[:],
                    in0=ssqs[:, sl],
                    scalar=1.0 / C,
                    in1=var8[:],
                    op0=ALU.mult,
                    op1=ALU.subtract,
                )
                _act_rsqrt(nc, rs32[:, sl], var8[:], eps_t[:])
                nc.vector.scalar_tensor_tensor(
                    out=nmr32[:, sl],
                    in0=mu32[:, sl],
                    scalar=-1.0,
                    in1=rs32[:, sl],
                    op0=ALU.mult,
                    op1=ALU.mult,
                )

            # =============== Phase C2: LN2 apply, MLP, out ==================
            g1_tiles = {}

            def c2_front(n):
                xh4c = work.tile([P, 4, C], BF16, tag="xh4", name=f"x2{n}")
                for s in range(4):
                    i = n * 4 + s
                    nc.vector.tensor_scalar(
                        out=xh4c[:, s, :],
                        in0=h1_sb[:, i, :],
                        scalar1=mu32[:, i : i + 1],
                        scalar2=rs32[:, i : i + 1],
                        op0=ALU.subtract,
                        op1=ALU.mult,
                    )
                x2tb = tpb.tile([P, 8, P], BF16, tag="x2tb")
                nc.sync.dma_start_transpose(out=x2tb[:], in_=xh4c[:])
                x2t8 = big.tile([P, 2, 512], FP8, tag="x2t8")
                nc.vector.tensor_copy(
                    out=x2t8[:].rearrange("p k (i t) -> p k i t", t=P),
                    in_=x2tb[:].rearrange("p (i k) t -> p k i t", k=2),
                )
                g1T8 = big.tile([P, 8, 512], FP8, tag="g1T8")
                g1_tiles[n] = g1T8
                for hc in range(8):
                    f_ps = ps_big.tile([P, 512], F32, tag="big")
                    nc.tensor.matmul(
                        f_ps[:],
                        w1[:, :, hc * P : (hc + 1) * P],
                        x2t8[:],
                        start=True,
                        stop=True,
                        perf_mode=DR,
                    )
                    nc.scalar.activation(
                        out=g1T8[:, hc, :],
                        in_=f_ps[:],
                        func=AF.Gelu,
                        bias=biases[:, 2 + hc : 3 + hc],
                        scale=1.0 / WS,
                    )

            def c2_back(n):
                g1T8 = g1_tiles.pop(n)
                o_sb = outp.tile([P, 4, C], F32, tag="osb")
                for s in range(4):
                    i = n * 4 + s
                    m_ps = ps_small.tile([P, C], F32, tag="small", name=f"mp{n}_{s}")
                    for j in range(4):
                        nc.tensor.matmul(
                            m_ps[:],
                            g1T8[:, 2 * j : 2 * j + 2, s * P : (s + 1) * P],
                            w2[:, 2 * j : 2 * j + 2, :],
                            start=(j == 0),
                            stop=(j == 3 and not has_bfc2),
                            perf_mode=DR,
                        )
                    if has_bfc2:
                        nc.tensor.matmul(
                            m_ps[:], ones_row[:], bfc2[:], start=False, stop=True
                        )
                    nc.vector.scalar_tensor_tensor(
                        out=o_sb[:, s, :],
                        in0=m_ps[:],
                        scalar=1.0 / WS,
                        in1=x_sb[:, i, :],
                        op0=ALU.mult,
                        op1=ALU.add,
                    )
                nc.sync.dma_start(
                    out=out_d[:, n * 4 : (n + 1) * 4, :], in_=o_sb[:]
                )

            for i in range(16):
                c1_tile(i)
            rstd8(0)
            c2_front(0)
            for i in range(16, 24):
                c1_tile(i)
            c2_front(1)
            c2_back(0)
            for i in range(24, 32):
                c1_tile(i)
            rstd8(1)
            for n_ in range(2, 8):
                c2_front(n_)
                c2_back(n_ - 1)
            c2_back(7)

    return nc


def _finalize_nc(nc):
    """Walrus-only lowering fixups (applied once, after any CoreSim use)."""
    if not getattr(nc, "_sync_waits_split", False):
        _split_sync_waits(nc)
        nc._sync_waits_split = True
    return nc


_CACHE = {}


def _get_nc(key):
    if key not in _CACHE:
        _CACHE[key] = _build_nc(*key)
    return _CACHE[key]


def _prep_inputs(inputs):
    f32 = lambda k: np.asarray(inputs[k], dtype=np.float32)
    qkv_w, qkv_b = f32("qkv_w"), f32("qkv_b")
    proj_w, proj_b = f32("proj_w"), f32("proj_b")
    ln1_g, ln1_b = f32("ln1_g"), f32("ln1_b")
    ln2_g, ln2_b = f32("ln2_g"), f32("ln2_b")
    fc1_w, fc1_b = f32("fc1_w"), f32("fc1_b")
    fc2_w, fc2_b = f32("fc2_w"), f32("fc2_b")

    # Fold LN affines into the following matmuls
    wqkv_f = ln1_g[:, None] * qkv_w
    bqkv_f = ln1_b @ qkv_w + qkv_b
    w1_f = ln2_g[:, None] * fc1_w
    b1_f = ln2_b @ fc1_w + fc1_b

    wq = wqkv_f[:, 0:C]  # [c_in, f]
    wkv = wqkv_f[:, C : 3 * C]  # [c_in, 2C]
    bq = bqkv_f[0:C]
    bkv = bqkv_f[C : 3 * C]

    has_flags = (
        bool(np.any(bkv != 0)),
        bool(np.any(proj_b != 0)),
        bool(np.any(bq != 0)),
        bool(np.any(fc2_b != 0)),
    )

    # bf16 blob: wproj [2,128,256] | wqT [2,128,256]
    wproj_r = proj_w.reshape(2, P, C).transpose(1, 0, 2).reshape(P, 512)
    wqT_r = wq.T.reshape(2, P, C).transpose(1, 0, 2).reshape(P, 512)
    wbf = np.concatenate([wproj_r, wqT_r], axis=1).astype(NP_BF16)

    # fp8 blob (x WS): wkv | w1 | w2
    wkv_r = (wkv * WS).reshape(2, P, 2 * C).transpose(1, 0, 2).reshape(P, 1024)
    w1_r = (w1_f * WS).reshape(2, P, HID).transpose(1, 0, 2).reshape(P, 2048)
    w2_r = (fc2_w * WS).reshape(8, P, C).transpose(1, 0, 2).reshape(P, 2048)
    w8 = np.concatenate([wkv_r, w1_r, w2_r], axis=1).astype(NP_FP8)

    bias = np.concatenate(
        [bq.reshape(2, P).T, b1_f.reshape(8, P).T, fc2_b.reshape(2, P).T],
        axis=1,
    ).astype(np.float32)

    shared = {
        "wbf": wbf,
        "w8": w8,
        "bias": bias,
        "bkv": (bkv * WS).reshape(1, 2 * C).astype(NP_BF16),
        "bproj": (proj_b * FS).reshape(1, C).astype(NP_BF16),
        "bfc2": (fc2_b * WS).reshape(1, C).astype(NP_BF16),
    }
    return shared, has_flags


def kernel(x, **weights):
    x = np.asarray(x, dtype=np.float32)
    shared, has_flags = _prep_inputs(weights)
    nc = _finalize_nc(_get_nc(has_flags))
    in_maps = [
        dict(shared, x=np.ascontiguousarray(x[b].reshape(P, NTILES, C)).astype(NP_BF16))
        for b in range(B)
    ]
    res = run_bass_kernel_spmd(nc, in_maps, list(range(B)))
    out = np.stack(
        [res.results[b]["out"].reshape(N, C) for b in range(B)], axis=0
    )
    return out.astype(np.float32)
